# revision 24
# baseline (speedup 1.0000x reference)
"""Multi-head attention (B=2, S=4096, D=768, H=12, hd=64) on 8 trn2 NeuronCores.

Sharding: core c -> batch b = c//4, heads [3*(c%4), 3*(c%4)+3)  (batch- and
head-parallel; no device collectives).  Each core computes the partial
output  sum_h softmax((x Wq_h + bq_h)(x Wk_h + bk_h)^T / 8) (x Wv_h) Wo_h
for its 3 heads as a full [S, 768] tensor (bf16); the host sums the 4
partials per batch in f32 and adds the bias terms (bo + bv @ Wo, since
softmax rows sum to 1).

Per-core device algorithm (all matmuls bf16, f32 psum accumulate).  The
engines execute their queues in order, and ACT (the exp engine, ~1
elem/lane/cycle on 3*S*S scores) is the critical resource at ~390us
busy -- the structure below keeps ACT fed:

  - host ships x[b]^T chunk-major as [S/512, 128, 6*512] so each 512-col
    chunk of all 6 d-blocks is ONE DMA; weights are packed the same way
  - q/k projections emitted per chunk (blk0 = [q0 q1], blk1 = [k0 k1]),
    interleaved with the hoisted (h0, qc0) score groups so the first exp
    lands within a few us of the first x chunk; each head's 64 d-rows
    are DMA-replicated to the other partition half (only DMA can shift
    partitions)
  - scores computed transposed as K=64 matmuls: ST[k-block, q-chunk] =
    kT^T q with tile_position=(0,0) for even k-blocks (lo half) and
    (64,0) for odd ones (hi half); adjacent matmuls hit different PE
    row-groups and execute concurrently (measured dt_start ~3ns),
    recovering the utilization lost to hd=64 < 128.  exp on ACT (no max
    subtraction: |scores/8| <~ 2 for this problem)
  - attn_out^T accumulated directly: acc[0:64, q] = sum_k V[k,:]^T P^T[k, q],
    row 64 = sum_k exp (the ones column); no output transpose is needed
  - unit pipeline over (qc, h): each unit's P@V + normalize are DEFERRED
    into the next unit's group loop, so score matmuls (which gate exp)
    are never queued behind a PV batch; proj_v / blk2 / fin work is
    injected per-group the same way.  The last three units run PV inline
    (lag 1 group) so the tail stays short.
  - PSUM: 6 banks of double-buffered score tiles, 1 bank for the live
    P@V accumulator, 1 bank shared by all short-lived projection /
    final-projection tiles (manually alternated 256-col halves; the
    framework tracks dependencies per region)
  - normalized with reciprocal_approx_fast of the exp-sum row (~5x
    faster than exact; denominators are O(1e3) so approx is safe),
    partition-broadcast via a DRAM round trip, then the final projection
    against Wo rows (K=128 with zero padding: mixed tile_position row
    offsets in one accumulation group crash the device)
"""

import numpy as np
from contextlib import ExitStack

import concourse.bass as bass
import concourse.bacc as bacc
import concourse.mybir as mybir
from concourse import tile

BF16 = mybir.dt.bfloat16
F32 = mybir.dt.float32
AF = mybir.ActivationFunctionType

D_MODEL = 768
N_HEADS = 12
HD = 64
N_CORES = 8
NH_LOC = 3          # heads per core
DC = D_MODEL // 128  # 6 chunks of d_model
CHUNK = 512          # q columns processed per score chunk
GRP = 3              # k-blocks (of 128) per psum score tile / exp call
PAIR = True          # K=64 row-tiled score matmuls (vs K=128 zero-padded)


def build(nc, S, level=3):
    """Emit the per-core program (SPMD; all cores run this with their shard).

    level: debug knob — 1 = projections only, 2 = + attention, 3 = full.
    """
    SB = S // 128     # seq blocks of 128
    NCH = S // CHUNK  # q chunks
    KB = S // 128     # k blocks of 128
    NXH = S // 512    # x column chunks

    xT_d = nc.declare_dram_parameter("xT", [NXH, 128, DC * 512], BF16,
                                     isOutput=False)
    wqk_d = nc.declare_dram_parameter("wqk", [3, 128, DC * 128], BF16,
                                      isOutput=False)
    bqk_d = nc.declare_dram_parameter("bqk", [128, 3], F32, isOutput=False)
    wv_d = nc.declare_dram_parameter("wv", [128, DC * NH_LOC * HD], BF16,
                                     isOutput=False)
    wo_d = nc.declare_dram_parameter("wo", [128, NH_LOC * D_MODEL], BF16,
                                     isOutput=False)
    out_d = nc.declare_dram_parameter("out", [S, D_MODEL], BF16, isOutput=True)

    with tile.TileContext(nc) as tc, ExitStack() as ctx:
        const = ctx.enter_context(tc.tile_pool(name="const", bufs=1))

        def ctile(name, shape, dt):
            return const.tile(shape, dt, tag=name, name=name)

        # --- constants / long-lived tensors -------------------------------
        xts = [ctile(f"xt{c}", [128, DC * 512], BF16) for c in range(NXH)]

        def xth(dcc, off, ln):
            # [off, off+ln) of logical xT d-block dcc (ln within one chunk)
            t = xts[off // 512]
            lo = off % 512
            return t[:, dcc * 512 + lo: dcc * 512 + lo + ln]
        wqks = [ctile(f"wqk{i}", [128, DC * 128], BF16) for i in range(3)]
        bqks = ctile("bqk", [128, 3], F32)
        wvs = ctile("wv", [128, DC * NH_LOC * HD], BF16)
        wos = ctile("wo", [128, NH_LOC * D_MODEL], BF16)
        # merged [V|1] tile: k-block j at cols j*195 as [V0|1|V1|1|V2|1] so
        # one strided DVE copy per s-block fills all three heads
        v1s = ctile("v1", [128, 195 * KB], BF16)
        qts = [ctile(f"qt{i}", [128, S], BF16) for i in range(NH_LOC)]
        kts = [ctile(f"kt{i}", [128, S], BF16) for i in range(NH_LOC)]
        ats = [[ctile(f"at{i}_{qc}", [128, CHUNK], BF16)
                for qc in range(NCH)] for i in range(NH_LOC)]

        pt_pool = ctx.enter_context(tc.tile_pool(name="pt", bufs=12))
        outst_pool = ctx.enter_context(tc.tile_pool(name="outst", bufs=2))
        small_pool = ctx.enter_context(tc.tile_pool(name="small", bufs=2))
        rb_pool = ctx.enter_context(tc.tile_pool(name="rb", bufs=2))
        dram_pool = ctx.enter_context(tc.tile_pool(name="drs", bufs=3, space="DRAM"))
        # PSUM: 6 banks double-buffered score tiles + 1 bank for the live
        # P@V accumulator + 1 bank shared by all short-lived tiles
        ps_st = ctx.enter_context(tc.tile_pool(name="ps_st", bufs=2, space="PSUM"))
        ps_acc = ctx.enter_context(tc.tile_pool(name="ps_acc", bufs=1, space="PSUM"))
        ps_mi = ctx.enter_context(tc.tile_pool(name="ps_mi", bufs=1, space="PSUM"))

        def acctile(nm):
            return ps_acc.tile([128, 512], F32, tag="acc", name=nm)

        # ONE misc psum bank shared by all short-lived tiles: full-width
        # users serialize on it (write-after-read tracked per region);
        # proj_v's narrow tiles alternate the two 256-col halves.  The acc
        # bank doubles as the second projection buffer before the first
        # P@V accumulator exists.
        acc_bank = ps_acc.tile([128, 512], F32, tag="acc", name="acc_pre")
        mi_tile = ps_mi.tile([128, 512], F32, tag="mi", name="mi")
        mi_state = [0]

        def mitile(width, full=False):
            if full:
                return mi_tile[:, 0:width]
            h = mi_state[0]
            mi_state[0] ^= 1
            return mi_tile[:, h * 256: h * 256 + width]

        # --- load inputs ---------------------------------------------------
        # one DMA queue moves only ~115 GB/s, so the first x chunk is split
        # across the three DMA-capable queues (sync/scalar/gpsimd) and later
        # chunks round-robin; weights needed late load last.
        third = (DC // 3) * 512
        nc.scalar.dma_start(xts[0][:, 0:third], xT_d[0, :, 0:third])
        nc.gpsimd.dma_start(xts[0][:, third:2 * third],
                            xT_d[0, :, third:2 * third])
        nc.sync.dma_start(xts[0][:, 2 * third:], xT_d[0, :, 2 * third:])
        nc.sync.dma_start(wqks[0][:], wqk_d[0])
        nc.scalar.dma_start(wqks[1][:], wqk_d[1])
        nc.gpsimd.dma_start(bqks[:], bqk_d[:])
        qs = [nc.sync, nc.scalar, nc.gpsimd]
        for ch in range(1, NXH):
            qs[ch % 3].dma_start(xts[ch][:], xT_d[ch])
        nc.sync.dma_start(wqks[2][:], wqk_d[2])
        nc.scalar.dma_start(wvs[:], wv_d[:])
        nc.sync.dma_start(wos[:], wo_d[:])

        # hoist the ACT exp-table load (~2.7us) under the input DMAs: walrus
        # inserts the table load before the first ACTIVATE on the queue
        wrm_in = ctile("wrm_in", [128, 8], F32)
        wrm_out = ctile("wrm_out", [128, 8], F32)
        nc.vector.memset(wrm_in[:], 0.0)
        nc.scalar.activation(wrm_out[:], wrm_in[:], AF.Exp)
        # dummy matmul burst during the DMA wait: flips the PE HAM clock
        # gate to 8/8 (~3.4us of sustained activity) so the first real
        # projections run at 2.4 GHz instead of 1.2
        dmy = ctile("dmy", [128, 512], BF16)
        nc.vector.memset(dmy[:], 0.0)
        for _ in range(12):
            nc.tensor.matmul(mi_tile[:], lhsT=dmy[:, 0:128],
                             rhs=dmy[:], start=True, stop=True)

        # --- phase 1: projections -----------------------------------------
        def repl(dst, lo_src):
            # replicate a 64-partition half to the other half (DMA only)
            if PAIR:
                nc.gpsimd.dma_start(dst, lo_src)

        def proj_chunk(blk, sc, pp):
            # qT / kT block: [d_out(128 part), s] = W_blk^T x^T
            # blk0 = [q0 q1] -> Q0 rows 0:64 / Q1 rows 64:128
            # blk1 = [k0 k1] -> K0 / K1
            # blk2 = [q2 k2] -> Q2 rows 0:64; k2 rows 64:128 (bias-added in
            #   place, partitions match the psum half)
            # pp: [128, 512] psum region supplied by the caller
            for dcc in range(DC):
                nc.tensor.matmul(
                    pp,
                    lhsT=wqks[blk][:, dcc * 128:(dcc + 1) * 128],
                    rhs=xth(dcc, sc * 512, 512),
                    start=(dcc == 0),
                    stop=(dcc == DC - 1),
                )
            sl = slice(sc * 512, (sc + 1) * 512)
            if blk == 0 or blk == 1:
                dsts = qts if blk == 0 else kts
                nc.vector.tensor_scalar_add(
                    dsts[0][0:64, sl], pp[0:64, :], bqks[0:64, blk:blk + 1])
                nc.vector.tensor_scalar_add(
                    dsts[1][64:128, sl], pp[64:128, :],
                    bqks[64:128, blk:blk + 1])
                repl(dsts[0][64:128, sl], dsts[0][0:64, sl])
                repl(dsts[1][0:64, sl], dsts[1][64:128, sl])
            else:
                nc.vector.tensor_scalar_add(
                    qts[2][0:64, sl], pp[0:64, :], bqks[0:64, 2:3])
                nc.vector.tensor_scalar_add(
                    kts[2][64:128, sl], pp[64:128, :], bqks[64:128, 2:3])
                repl(qts[2][64:128, sl], qts[2][0:64, sl])
                # k2 lo half is needed even without PAIR
                nc.gpsimd.dma_start(kts[2][0:64, sl], kts[2][64:128, sl])

        def proj_v_unit(sb):
            # V in [s, d] layout; one strided copy drops all 3 heads into
            # the merged [V0|1|V1|1|V2|1] block (ones pre-set by memset)
            pv = mitile(NH_LOC * HD)
            for dcc in range(DC):
                nc.tensor.matmul(
                    pv,
                    lhsT=xth(dcc, sb * 128, 128),
                    rhs=wvs[:, (dcc * NH_LOC) * HD:(dcc * NH_LOC + NH_LOC) * HD],
                    start=(dcc == 0),
                    stop=(dcc == DC - 1),
                )
            dst = v1s[:, sb * 195: sb * 195 + 195].rearrange(
                "p (h c) -> p h c", h=NH_LOC)[:, :, 0:HD]
            nc.vector.tensor_copy(
                dst, pv[:, 0:NH_LOC * HD].rearrange("p (h c) -> p h c",
                                                    h=NH_LOC))

        if level < 2:
            for sc in range(NXH):
                proj_chunk(0, sc, acc_bank[:] if sc % 2 else mi_tile[:])
                proj_chunk(1, sc, mi_tile[:] if sc % 2 else acc_bank[:])
                proj_chunk(2, sc, acc_bank[:] if sc % 2 else mi_tile[:])
            nc.vector.memset(v1s[:], 1.0)
            for sb in range(SB):
                proj_v_unit(sb)
            for sb in range(SB):
                ost = outst_pool.tile([128, D_MODEL], BF16, tag="ost",
                                      name=f"ost{sb}")
                nc.vector.memset(ost[:], 0.0)
                nc.sync.dma_start(out_d[sb * 128:(sb + 1) * 128, :], ost[:])
            return nc

        # --- phase 2+3: attention -----------------------------------------
        groups = []
        j0 = 0
        while j0 < KB:
            groups.append((j0, min(GRP, KB - j0)))
            j0 += GRP
        NG = len(groups)

        def phase_a(h, qc, g0, glen):
            qt, kt = qts[h], kts[h]
            st = ps_st.tile([128, GRP * CHUNK], F32, tag="st",
                            name=f"st{h}_{qc}_{g0}")
            for t in range(glen):
                j = g0 + t
                if PAIR:
                    # K=64: even k-blocks read the lo partition half at PE
                    # rows 0:64, odd ones the replicated hi half at rows
                    # 64:128 -> adjacent matmuls overlap in the array
                    r = 64 * (j % 2)
                    nc.tensor.matmul(
                        st[:, t * CHUNK:(t + 1) * CHUNK],
                        lhsT=kt[r:r + 64, j * 128:(j + 1) * 128],
                        rhs=qt[r:r + 64, qc * CHUNK:(qc + 1) * CHUNK],
                        start=True,
                        stop=True,
                        tile_position=(r, 0),
                    )
                else:
                    nc.tensor.matmul(
                        st[:, t * CHUNK:(t + 1) * CHUNK],
                        lhsT=kt[:, j * 128:(j + 1) * 128],
                        rhs=qt[:, qc * CHUNK:(qc + 1) * CHUNK],
                        start=True,
                        stop=True,
                    )
            pt = pt_pool.tile([128, GRP * CHUNK], BF16, tag="pt",
                              name=f"pt{h}_{qc}_{g0}")
            nc.scalar.activation(
                pt[:, 0:glen * CHUNK],
                st[:, 0:glen * CHUNK],
                AF.Exp,
                scale=0.125,
            )
            return pt

        def fin_unit(qc, sb, bank=None):
            # final projection for one 128-row s-block of chunk qc (deferred
            # so the normalize round trip is off the critical path); its two
            # psum tiles serialize on the misc bank (fin has PE slack), or on
            # `bank` when given (tail fins run on two banks in parallel).
            # NOTE: all accumulating matmuls in one psum group must share one
            # tile_position (mixed row offsets crash the device), so every
            # head's AT / Wo tile lives at partition offset 0 with the K=128
            # zero padding.
            ost = outst_pool.tile([128, D_MODEL], BF16, tag="ost",
                                  name=f"ost{sb}")
            for (n0, n1) in ((0, 512), (512, D_MODEL)):
                po = bank[:, 0:n1 - n0] if bank is not None else \
                    mitile(n1 - n0, full=True)
                sb_in = sb % (CHUNK // 128)
                for h in range(NH_LOC):
                    nc.tensor.matmul(
                        po,
                        lhsT=ats[h][qc][:, sb_in * 128:(sb_in + 1) * 128],
                        rhs=wos[:, h * D_MODEL + n0:h * D_MODEL + n1],
                        start=(h == 0),
                        stop=(h == NH_LOC - 1),
                    )
                nc.vector.tensor_copy(ost[:, n0:n1], po)
            nc.gpsimd.dma_start(out_d[sb * 128:(sb + 1) * 128, :], ost[:])

        def pv_mms(h, acc, pt, g0, glen):
            # acc[d, q] = sum_k [V|1][k,:]^T exp(ST)[k, q]:
            # rows 0..63 = attn_out^T (unnormalized), row 64 = sum(exp)
            for t in range(glen):
                j = g0 + t
                nc.tensor.matmul(
                    acc[0:65, :],
                    lhsT=v1s[:, j * 195 + h * 65: j * 195 + h * 65 + 65],
                    rhs=pt[:, t * CHUNK:(t + 1) * CHUNK],
                    start=(j == 0),
                    stop=(j == KB - 1),
                )

        def normalize(h, qc, acc):
            # copy the accumulator off psum (frees the acc bank), broadcast
            # the sum row to partitions 0..63 via a DRAM round trip (DMA
            # can't read PSUM; SBUF APs can't have a zero partition step;
            # DVE can't shift partitions).
            tmp = small_pool.tile([65, CHUNK], F32, tag="r1",
                                  name=f"r1_{h}_{qc}")
            nc.vector.tensor_copy(tmp[:], acc[0:65, :])
            drs = dram_pool.tile([1, CHUNK], F32, tag="drs",
                                 name=f"drs{h}_{qc}")
            nc.sync.dma_start(drs[:], tmp[64:65, :])
            rbs = rb_pool.tile([HD, CHUNK], F32, tag="rbs",
                               name=f"rbs{h}_{qc}")
            nc.sync.dma_start(rbs[:], drs[:].to_broadcast([HD, CHUNK]))
            rbr = rb_pool.tile([HD, CHUNK], F32, tag="rbr",
                               name=f"rbr{h}_{qc}")
            nc.vector.reciprocal_approx_fast(rbr[:], rbs[:])
            nc.vector.tensor_mul(
                ats[h][qc][0:HD, :],
                tmp[0:HD, :],
                rbr[:],
            )

        # interleave q/k projection chunks with the hoisted (h0, qc0) score
        # groups: group g touches k-blocks 3g..3g+2 -> ready after chunk
        # (3g+2)//4 (and q-chunk 0).  ACT starts exp'ing within a few us.
        hoist_after = {}  # chunk -> list of group indices
        for gi, (g0, glen) in enumerate(groups):
            need = (g0 + glen - 1) // 4
            hoist_after.setdefault(need, []).append(gi)
        # proj01 double-buffers across the misc bank and the (not yet used)
        # P@V accumulator bank
        pts0 = [None] * NG
        for sc in range(NXH):
            proj_chunk(0, sc, mi_tile[:])
            proj_chunk(1, sc, acc_bank[:])
            for gi in hoist_after.get(sc, []):
                g0, glen = groups[gi]
                pts0[gi] = phase_a(0, 0, g0, glen)
        # ones columns (DVE: keeps gpsimd free for the replication DMAs)
        nc.vector.memset(v1s[:], 1.0)

        # --- unit pipeline: unit u = (qc, h); unit 0's scores are hoisted
        # above.  Units 1..NU-4: the previous unit's P@V + normalize run
        # inside this unit's group loop (deferral).  Unit NU-3 catches up
        # the deferred unit then switches to inline; the last two units run
        # inline with a 1-group lag (acc bank ping-pongs cleanly).
        units = [(qc, h) for qc in range(NCH) for h in range(NH_LOC)]
        NU = len(units)

        def extras_pre(u):
            if u == 1:
                # seed proj_v one group ahead of PV(unit 0)'s v1s reads
                for sb in range(3):
                    proj_v_unit(sb)

        def extras(u, gi):
            # blk2 ([q2 k2]) projections serialize on the misc bank; start
            # them at unit 1's tail so unit 2's first scores don't wait
            qc, h = units[u]
            if u == 1:  # proj_v skewed +1 group ahead of PV(unit 0)
                for sb in range(3 * gi + 3, min(3 * gi + 6, SB)):
                    proj_v_unit(sb)
                if gi == NG - 2:
                    for sb in range(3 * NG, SB):
                        proj_v_unit(sb)
                if gi >= NG - 2:
                    proj_chunk(2, gi - (NG - 2), mi_tile[:])
            elif u == 2:
                if gi < NXH - 2:
                    proj_chunk(2, gi + 2, mi_tile[:])
                if gi == 0:
                    # fin zero padding (needed first by fin(qc0) at unit 4)
                    for hh in range(NH_LOC):
                        for qq in range(NCH):
                            nc.vector.memset(ats[hh][qq][HD:128, :], 0.0)
            elif qc > 0 and h == 1 and gi < CHUNK // 128 and level >= 3:
                fin_unit(qc - 1, (qc - 1) * (CHUNK // 128) + gi)

        prev_pts, prev_unit = pts0, 0
        for u in range(1, NU - 2):
            qc, h = units[u]
            pqc, ph = units[prev_unit]
            prev_acc = acctile(f"acc{ph}_{pqc}")
            extras_pre(u)
            cur_pts = [None] * NG
            for gi, (g0, glen) in enumerate(groups):
                extras(u, gi)
                pv_mms(ph, prev_acc, prev_pts[gi], g0, glen)
                cur_pts[gi] = phase_a(h, qc, g0, glen)
            normalize(ph, pqc, prev_acc)
            prev_pts, prev_unit = cur_pts, u

        # unit NU-2: catch up the deferred unit NU-3, then inline
        u = NU - 2
        qc, h = units[u]
        pqc, ph = units[prev_unit]
        prev_acc = acctile(f"acc{ph}_{pqc}")
        acc = None
        cur_pts = [None] * NG
        for gi, (g0, glen) in enumerate(groups):
            # catch-up: 2 groups of the deferred unit per group here
            for cg in (2 * gi, 2 * gi + 1):
                if cg < NG:
                    cg0, cglen = groups[cg]
                    pv_mms(ph, prev_acc, prev_pts[cg], cg0, cglen)
            if gi == (NG + 1) // 2:
                normalize(ph, pqc, prev_acc)
                acc = acctile(f"acc{h}_{qc}")
            extras(u, gi)
            cur_pts[gi] = phase_a(h, qc, g0, glen)
            if acc is not None and gi >= (NG + 1) // 2 + 1:
                lg = gi - 1 - (NG + 1) // 2
                g0l, glenl = groups[lg]
                pv_mms(h, acc, cur_pts[lg], g0l, glenl)
        for lg in range(NG - 1 - (NG + 1) // 2, NG):
            g0l, glenl = groups[lg]
            pv_mms(h, acc, cur_pts[lg], g0l, glenl)
        normalize(h, qc, acc)

        # last unit: inline with 1-group lag
        u = NU - 1
        qc, h = units[u]
        acc = acctile(f"acc{h}_{qc}")
        cur_pts = [None] * NG
        for gi, (g0, glen) in enumerate(groups):
            extras(u, gi)
            cur_pts[gi] = phase_a(h, qc, g0, glen)
            if gi >= 1:
                g0l, glenl = groups[gi - 1]
                pv_mms(h, acc, cur_pts[gi - 1], g0l, glenl)
        g0l, glenl = groups[NG - 1]
        pv_mms(h, acc, cur_pts[NG - 1], g0l, glenl)
        normalize(h, qc, acc)

        if level < 3:
            for sb in range(SB):
                ost = outst_pool.tile([128, D_MODEL], BF16, tag="ost",
                                      name=f"ost{sb}")
                nc.vector.memset(ost[:], 0.0)
                nc.sync.dma_start(out_d[sb * 128:(sb + 1) * 128, :], ost[:])
            return nc
        fin_acc = acctile("fin_acc")
        for i, sb in enumerate(range((NCH - 1) * (CHUNK // 128),
                                     NCH * (CHUNK // 128))):
            fin_unit(NCH - 1, sb, bank=fin_acc[:] if i % 2 else None)

    return nc


def make_nc(S=4096, level=3):
    nc = bacc.Bacc(None, target_bir_lowering=False, debug=False)
    build(nc, S, level=level)
    nc.compile()
    return nc


def shard_inputs(x, Wq, bq, Wk, bk, Wv, bv, Wo, bo, S):
    """Host-side packing of the 8 per-core input maps (bf16 casts included)."""
    import ml_dtypes

    bf = ml_dtypes.bfloat16
    NXH = S // 512
    in_maps = []
    for c in range(N_CORES):
        b = c // 4
        h0 = NH_LOC * (c % 4)
        cs, ce = h0 * HD, (h0 + NH_LOC) * HD
        # chunk-major xT: [NXH, 128, DC*512]; chunk ch holds columns
        # [ch*512,(ch+1)*512) of all DC d-blocks side by side
        xt = np.ascontiguousarray(x[b].T).astype(bf).reshape(DC, 128, S)
        xT = np.ascontiguousarray(
            xt.reshape(DC, 128, NXH, 512).transpose(2, 1, 0, 3)
        ).reshape(NXH, 128, DC * 512)

        def pack_w(w2):  # [768, n] -> [128, DC*n] (d-blocks side by side)
            n = w2.shape[1]
            return np.ascontiguousarray(
                w2.reshape(DC, 128, n).transpose(1, 0, 2)
            ).astype(bf).reshape(128, DC * n)

        wqk = np.stack([
            pack_w(Wq[:, cs:cs + 2 * HD]),
            pack_w(Wk[:, cs:cs + 2 * HD]),
            pack_w(np.concatenate([Wq[:, cs + 2 * HD:ce],
                                   Wk[:, cs + 2 * HD:ce]], axis=1)),
        ])
        bqk = np.stack([
            bq[cs:cs + 2 * HD],
            bk[cs:cs + 2 * HD],
            np.concatenate([bq[cs + 2 * HD:ce], bk[cs + 2 * HD:ce]]),
        ], axis=1).astype(np.float32)  # [128, 3]
        wv = pack_w(Wv[:, cs:ce])
        wo = np.zeros((NH_LOC, 128, D_MODEL), np.float32)
        wo[:, 0:HD, :] = Wo[cs:ce, :].reshape(NH_LOC, HD, D_MODEL)
        wo = np.ascontiguousarray(wo.transpose(1, 0, 2)).astype(bf).reshape(
            128, NH_LOC * D_MODEL)
        in_maps.append({"xT": xT, "wqk": wqk, "bqk": bqk, "wv": wv, "wo": wo})
    return in_maps


_NC_CACHE = {}


def kernel(x, Wq, bq, Wk, bk, Wv, bv, Wo, bo):
    from concourse import bass_utils

    x = np.asarray(x, np.float32)
    Wq, bq = np.asarray(Wq, np.float32), np.asarray(bq, np.float32)
    Wk, bk = np.asarray(Wk, np.float32), np.asarray(bk, np.float32)
    Wv, bv = np.asarray(Wv, np.float32), np.asarray(bv, np.float32)
    Wo, bo = np.asarray(Wo, np.float32), np.asarray(bo, np.float32)
    B, S, D = x.shape
    assert (B, D) == (2, D_MODEL)
    if S not in _NC_CACHE:
        _NC_CACHE[S] = make_nc(S)
    nc = _NC_CACHE[S]

    in_maps = shard_inputs(x, Wq, bq, Wk, bk, Wv, bv, Wo, bo, S)
    res = bass_utils.run_bass_kernel_spmd(nc, in_maps, core_ids=list(range(N_CORES)))

    # host reduction: sum head-group partials per batch, add bias terms
    bias = (bo.astype(np.float32)
            + bv.astype(np.float32) @ Wo.astype(np.float32))  # [768]
    out = np.empty((B, S, D_MODEL), np.float32)
    for b in range(B):
        acc = res.results[4 * b]["out"].astype(np.float32)
        for c in range(4 * b + 1, 4 * b + 4):
            acc += res.results[c]["out"].astype(np.float32)
        out[b] = acc + bias
    return out


# revision 25
# speedup vs baseline: 1.0274x; 1.0274x over previous
"""Multi-head attention (B=2, S=4096, D=768, H=12, hd=64) on 8 trn2 NeuronCores.

Sharding: core c -> batch b = c//4, heads [3*(c%4), 3*(c%4)+3)  (batch- and
head-parallel; no device collectives).  Each core computes the partial
output  sum_h softmax((x Wq_h + bq_h)(x Wk_h + bk_h)^T / 8) (x Wv_h) Wo_h
for its 3 heads as a full [S, 768] tensor (bf16); the host sums the 4
partials per batch in f32 and adds the bias terms (bo + bv @ Wo, since
softmax rows sum to 1).

Per-core device algorithm (all matmuls bf16, f32 psum accumulate).  The
engines execute their queues in order, and ACT (the exp engine, ~1
elem/lane/cycle on 3*S*S scores) is the critical resource at ~390us
busy -- the structure below keeps ACT fed:

  - host ships x[b]^T chunk-major as [S/512, 128, 6*512] so each 512-col
    chunk of all 6 d-blocks is ONE DMA; weights are packed the same way
  - q/k projections emitted per chunk (blk0 = [q0 q1], blk1 = [k0 k1]),
    interleaved with the hoisted (h0, qc0) score groups so the first exp
    lands within a few us of the first x chunk; each head's 64 d-rows
    are DMA-replicated to the other partition half (only DMA can shift
    partitions)
  - scores computed transposed as K=64 matmuls: ST[k-block, q-chunk] =
    kT^T q with tile_position=(0,0) for even k-blocks (lo half) and
    (64,0) for odd ones (hi half); adjacent matmuls hit different PE
    row-groups and execute concurrently (measured dt_start ~3ns),
    recovering the utilization lost to hd=64 < 128.  exp on ACT (no max
    subtraction: |scores/8| <~ 2 for this problem)
  - attn_out^T accumulated directly: acc[0:64, q] = sum_k V[k,:]^T P^T[k, q],
    row 64 = sum_k exp (the ones column); no output transpose is needed
  - unit pipeline over (qc, h): each unit's P@V + normalize are DEFERRED
    into the next unit's group loop, so score matmuls (which gate exp)
    are never queued behind a PV batch; proj_v / blk2 / fin work is
    injected per-group the same way.  The last three units run PV inline
    (lag 1 group) so the tail stays short.
  - PSUM: 6 banks of double-buffered score tiles, 1 bank for the live
    P@V accumulator, 1 bank shared by all short-lived projection /
    final-projection tiles (manually alternated 256-col halves; the
    framework tracks dependencies per region)
  - normalized with reciprocal_approx_fast of the exp-sum row (~5x
    faster than exact; denominators are O(1e3) so approx is safe),
    partition-broadcast via a DRAM round trip, then the final projection
    against Wo rows (K=128 with zero padding: mixed tile_position row
    offsets in one accumulation group crash the device)
"""

import numpy as np
from contextlib import ExitStack

import concourse.bass as bass
import concourse.bacc as bacc
import concourse.mybir as mybir
from concourse import tile

BF16 = mybir.dt.bfloat16
F32 = mybir.dt.float32
AF = mybir.ActivationFunctionType

D_MODEL = 768
N_HEADS = 12
HD = 64
N_CORES = 8
NH_LOC = 3          # heads per core
DC = D_MODEL // 128  # 6 chunks of d_model
CHUNK = 512          # q columns processed per score chunk
GRP = 3              # k-blocks (of 128) per psum score tile / exp call
PAIR = True          # K=64 row-tiled score matmuls (vs K=128 zero-padded)


def build(nc, S, level=3):
    """Emit the per-core program (SPMD; all cores run this with their shard).

    level: debug knob — 1 = projections only, 2 = + attention, 3 = full.
    """
    SB = S // 128     # seq blocks of 128
    NCH = S // CHUNK  # q chunks
    KB = S // 128     # k blocks of 128
    NXH = S // 512    # x column chunks

    xT_d = nc.declare_dram_parameter("xT", [NXH, 128, DC * 512], BF16,
                                     isOutput=False)
    wqk_d = nc.declare_dram_parameter("wqk", [3, 128, DC * 128], BF16,
                                      isOutput=False)
    bqk_d = nc.declare_dram_parameter("bqk", [128, 3], F32, isOutput=False)
    wv_d = nc.declare_dram_parameter("wv", [128, DC * NH_LOC * HD], BF16,
                                     isOutput=False)
    wo_d = nc.declare_dram_parameter("wo", [128, NH_LOC * D_MODEL], BF16,
                                     isOutput=False)
    out_d = nc.declare_dram_parameter("out", [S, D_MODEL], BF16, isOutput=True)

    with tile.TileContext(nc) as tc, ExitStack() as ctx:
        const = ctx.enter_context(tc.tile_pool(name="const", bufs=1))

        def ctile(name, shape, dt):
            return const.tile(shape, dt, tag=name, name=name)

        # --- constants / long-lived tensors -------------------------------
        xts = [ctile(f"xt{c}", [128, DC * 512], BF16) for c in range(NXH)]

        def xth(dcc, off, ln):
            # [off, off+ln) of logical xT d-block dcc (ln within one chunk)
            t = xts[off // 512]
            lo = off % 512
            return t[:, dcc * 512 + lo: dcc * 512 + lo + ln]
        wqks = [ctile(f"wqk{i}", [128, DC * 128], BF16) for i in range(3)]
        bqks = ctile("bqk", [128, 3], F32)
        wvs = ctile("wv", [128, DC * NH_LOC * HD], BF16)
        wos = ctile("wo", [128, NH_LOC * D_MODEL], BF16)
        # merged [V|1] tile: k-block j at cols j*195 as [V0|1|V1|1|V2|1] so
        # one strided DVE copy per s-block fills all three heads
        v1s = ctile("v1", [128, 195 * KB], BF16)
        qts = [ctile(f"qt{i}", [128, S], BF16) for i in range(NH_LOC)]
        kts = [ctile(f"kt{i}", [128, S], BF16) for i in range(NH_LOC)]
        ats = [[ctile(f"at{i}_{qc}", [128, CHUNK], BF16)
                for qc in range(NCH)] for i in range(NH_LOC)]

        pt_pool = ctx.enter_context(tc.tile_pool(name="pt", bufs=12))
        outst_pool = ctx.enter_context(tc.tile_pool(name="outst", bufs=2))
        small_pool = ctx.enter_context(tc.tile_pool(name="small", bufs=2))
        rb_pool = ctx.enter_context(tc.tile_pool(name="rb", bufs=2))
        dram_pool = ctx.enter_context(tc.tile_pool(name="drs", bufs=3, space="DRAM"))
        # PSUM: 6 banks double-buffered score tiles + 1 bank for the live
        # P@V accumulator + 1 bank shared by all short-lived tiles
        ps_st = ctx.enter_context(tc.tile_pool(name="ps_st", bufs=2, space="PSUM"))
        ps_acc = ctx.enter_context(tc.tile_pool(name="ps_acc", bufs=1, space="PSUM"))
        ps_mi = ctx.enter_context(tc.tile_pool(name="ps_mi", bufs=1, space="PSUM"))

        def acctile(nm):
            return ps_acc.tile([128, 512], F32, tag="acc", name=nm)

        # ONE misc psum bank shared by all short-lived tiles: full-width
        # users serialize on it (write-after-read tracked per region);
        # proj_v's narrow tiles alternate the two 256-col halves.  The acc
        # bank doubles as the second projection buffer before the first
        # P@V accumulator exists.
        acc_bank = ps_acc.tile([128, 512], F32, tag="acc", name="acc_pre")
        mi_tile = ps_mi.tile([128, 512], F32, tag="mi", name="mi")
        mi_state = [0]

        def mitile(width, full=False):
            if full:
                return mi_tile[:, 0:width]
            h = mi_state[0]
            mi_state[0] ^= 1
            return mi_tile[:, h * 256: h * 256 + width]

        # --- load inputs ---------------------------------------------------
        # x chunks stream on the sync queue; the first two weight blocks go
        # over the scalar/gpsimd queues so the first projection can start
        # right after x chunk 0 lands.  Weights needed late load last.
        nc.scalar.dma_start(wqks[0][:], wqk_d[0])
        nc.gpsimd.dma_start(wqks[1][:], wqk_d[1])
        nc.gpsimd.dma_start(bqks[:], bqk_d[:])
        for ch in range(NXH):
            nc.sync.dma_start(xts[ch][:], xT_d[ch])
        nc.sync.dma_start(wqks[2][:], wqk_d[2])
        nc.sync.dma_start(wvs[:], wv_d[:])
        nc.sync.dma_start(wos[:], wo_d[:])

        # hoist the ACT exp-table load (~2.7us) under the input DMAs: walrus
        # inserts the table load before the first ACTIVATE on the queue
        wrm_in = ctile("wrm_in", [128, 8], F32)
        wrm_out = ctile("wrm_out", [128, 8], F32)
        nc.vector.memset(wrm_in[:], 0.0)
        nc.scalar.activation(wrm_out[:], wrm_in[:], AF.Exp)

        # --- phase 1: projections -----------------------------------------
        def repl(dst, lo_src):
            # replicate a 64-partition half to the other half (DMA only)
            if PAIR:
                nc.gpsimd.dma_start(dst, lo_src)

        def proj_chunk(blk, sc, pp):
            # qT / kT block: [d_out(128 part), s] = W_blk^T x^T
            # blk0 = [q0 q1] -> Q0 rows 0:64 / Q1 rows 64:128
            # blk1 = [k0 k1] -> K0 / K1
            # blk2 = [q2 k2] -> Q2 rows 0:64; k2 rows 64:128 (bias-added in
            #   place, partitions match the psum half)
            # pp: [128, 512] psum region supplied by the caller
            for dcc in range(DC):
                nc.tensor.matmul(
                    pp,
                    lhsT=wqks[blk][:, dcc * 128:(dcc + 1) * 128],
                    rhs=xth(dcc, sc * 512, 512),
                    start=(dcc == 0),
                    stop=(dcc == DC - 1),
                )
            sl = slice(sc * 512, (sc + 1) * 512)
            if blk == 0 or blk == 1:
                dsts = qts if blk == 0 else kts
                nc.vector.tensor_scalar_add(
                    dsts[0][0:64, sl], pp[0:64, :], bqks[0:64, blk:blk + 1])
                nc.vector.tensor_scalar_add(
                    dsts[1][64:128, sl], pp[64:128, :],
                    bqks[64:128, blk:blk + 1])
                repl(dsts[0][64:128, sl], dsts[0][0:64, sl])
                repl(dsts[1][0:64, sl], dsts[1][64:128, sl])
            else:
                nc.vector.tensor_scalar_add(
                    qts[2][0:64, sl], pp[0:64, :], bqks[0:64, 2:3])
                nc.vector.tensor_scalar_add(
                    kts[2][64:128, sl], pp[64:128, :], bqks[64:128, 2:3])
                repl(qts[2][64:128, sl], qts[2][0:64, sl])
                # k2 lo half is needed even without PAIR
                nc.gpsimd.dma_start(kts[2][0:64, sl], kts[2][64:128, sl])

        def proj_v_unit(sb):
            # V in [s, d] layout; one strided copy drops all 3 heads into
            # the merged [V0|1|V1|1|V2|1] block (ones pre-set by memset)
            pv = mitile(NH_LOC * HD)
            for dcc in range(DC):
                nc.tensor.matmul(
                    pv,
                    lhsT=xth(dcc, sb * 128, 128),
                    rhs=wvs[:, (dcc * NH_LOC) * HD:(dcc * NH_LOC + NH_LOC) * HD],
                    start=(dcc == 0),
                    stop=(dcc == DC - 1),
                )
            dst = v1s[:, sb * 195: sb * 195 + 195].rearrange(
                "p (h c) -> p h c", h=NH_LOC)[:, :, 0:HD]
            nc.vector.tensor_copy(
                dst, pv[:, 0:NH_LOC * HD].rearrange("p (h c) -> p h c",
                                                    h=NH_LOC))

        if level < 2:
            for sc in range(NXH):
                proj_chunk(0, sc, acc_bank[:] if sc % 2 else mi_tile[:])
                proj_chunk(1, sc, mi_tile[:] if sc % 2 else acc_bank[:])
                proj_chunk(2, sc, acc_bank[:] if sc % 2 else mi_tile[:])
            nc.vector.memset(v1s[:], 1.0)
            for sb in range(SB):
                proj_v_unit(sb)
            for sb in range(SB):
                ost = outst_pool.tile([128, D_MODEL], BF16, tag="ost",
                                      name=f"ost{sb}")
                nc.vector.memset(ost[:], 0.0)
                nc.sync.dma_start(out_d[sb * 128:(sb + 1) * 128, :], ost[:])
            return nc

        # --- phase 2+3: attention -----------------------------------------
        groups = []
        j0 = 0
        while j0 < KB:
            groups.append((j0, min(GRP, KB - j0)))
            j0 += GRP
        NG = len(groups)

        def phase_a(h, qc, g0, glen):
            qt, kt = qts[h], kts[h]
            st = ps_st.tile([128, GRP * CHUNK], F32, tag="st",
                            name=f"st{h}_{qc}_{g0}")
            for t in range(glen):
                j = g0 + t
                if PAIR:
                    # K=64: even k-blocks read the lo partition half at PE
                    # rows 0:64, odd ones the replicated hi half at rows
                    # 64:128 -> adjacent matmuls overlap in the array
                    r = 64 * (j % 2)
                    nc.tensor.matmul(
                        st[:, t * CHUNK:(t + 1) * CHUNK],
                        lhsT=kt[r:r + 64, j * 128:(j + 1) * 128],
                        rhs=qt[r:r + 64, qc * CHUNK:(qc + 1) * CHUNK],
                        start=True,
                        stop=True,
                        tile_position=(r, 0),
                    )
                else:
                    nc.tensor.matmul(
                        st[:, t * CHUNK:(t + 1) * CHUNK],
                        lhsT=kt[:, j * 128:(j + 1) * 128],
                        rhs=qt[:, qc * CHUNK:(qc + 1) * CHUNK],
                        start=True,
                        stop=True,
                    )
            pt = pt_pool.tile([128, GRP * CHUNK], BF16, tag="pt",
                              name=f"pt{h}_{qc}_{g0}")
            nc.scalar.activation(
                pt[:, 0:glen * CHUNK],
                st[:, 0:glen * CHUNK],
                AF.Exp,
                scale=0.125,
            )
            return pt

        def fin_unit(qc, sb, bank=None):
            # final projection for one 128-row s-block of chunk qc (deferred
            # so the normalize round trip is off the critical path); its two
            # psum tiles serialize on the misc bank (fin has PE slack), or on
            # `bank` when given (tail fins run on two banks in parallel).
            # NOTE: all accumulating matmuls in one psum group must share one
            # tile_position (mixed row offsets crash the device), so every
            # head's AT / Wo tile lives at partition offset 0 with the K=128
            # zero padding.
            ost = outst_pool.tile([128, D_MODEL], BF16, tag="ost",
                                  name=f"ost{sb}")
            for (n0, n1) in ((0, 512), (512, D_MODEL)):
                po = bank[:, 0:n1 - n0] if bank is not None else \
                    mitile(n1 - n0, full=True)
                sb_in = sb % (CHUNK // 128)
                for h in range(NH_LOC):
                    nc.tensor.matmul(
                        po,
                        lhsT=ats[h][qc][:, sb_in * 128:(sb_in + 1) * 128],
                        rhs=wos[:, h * D_MODEL + n0:h * D_MODEL + n1],
                        start=(h == 0),
                        stop=(h == NH_LOC - 1),
                    )
                nc.vector.tensor_copy(ost[:, n0:n1], po)
            nc.gpsimd.dma_start(out_d[sb * 128:(sb + 1) * 128, :], ost[:])

        def pv_mms(h, acc, pt, g0, glen):
            # acc[d, q] = sum_k [V|1][k,:]^T exp(ST)[k, q]:
            # rows 0..63 = attn_out^T (unnormalized), row 64 = sum(exp)
            for t in range(glen):
                j = g0 + t
                nc.tensor.matmul(
                    acc[0:65, :],
                    lhsT=v1s[:, j * 195 + h * 65: j * 195 + h * 65 + 65],
                    rhs=pt[:, t * CHUNK:(t + 1) * CHUNK],
                    start=(j == 0),
                    stop=(j == KB - 1),
                )

        def normalize(h, qc, acc):
            # copy the accumulator off psum (frees the acc bank), broadcast
            # the sum row to partitions 0..63 via a DRAM round trip (DMA
            # can't read PSUM; SBUF APs can't have a zero partition step;
            # DVE can't shift partitions).
            tmp = small_pool.tile([65, CHUNK], F32, tag="r1",
                                  name=f"r1_{h}_{qc}")
            nc.vector.tensor_copy(tmp[:], acc[0:65, :])
            drs = dram_pool.tile([1, CHUNK], F32, tag="drs",
                                 name=f"drs{h}_{qc}")
            nc.sync.dma_start(drs[:], tmp[64:65, :])
            rbs = rb_pool.tile([HD, CHUNK], F32, tag="rbs",
                               name=f"rbs{h}_{qc}")
            nc.sync.dma_start(rbs[:], drs[:].to_broadcast([HD, CHUNK]))
            rbr = rb_pool.tile([HD, CHUNK], F32, tag="rbr",
                               name=f"rbr{h}_{qc}")
            nc.vector.reciprocal_approx_fast(rbr[:], rbs[:])
            nc.vector.tensor_mul(
                ats[h][qc][0:HD, :],
                tmp[0:HD, :],
                rbr[:],
            )

        # interleave q/k projection chunks with the hoisted (h0, qc0) score
        # groups: group g touches k-blocks 3g..3g+2 -> ready after chunk
        # (3g+2)//4 (and q-chunk 0).  ACT starts exp'ing within a few us.
        hoist_after = {}  # chunk -> list of group indices
        for gi, (g0, glen) in enumerate(groups):
            need = (g0 + glen - 1) // 4
            hoist_after.setdefault(need, []).append(gi)
        # proj01 double-buffers across the misc bank and the (not yet used)
        # P@V accumulator bank
        pts0 = [None] * NG
        for sc in range(NXH):
            proj_chunk(0, sc, mi_tile[:])
            proj_chunk(1, sc, acc_bank[:])
            for gi in hoist_after.get(sc, []):
                g0, glen = groups[gi]
                pts0[gi] = phase_a(0, 0, g0, glen)
        # ones columns (DVE: keeps gpsimd free for the replication DMAs)
        nc.vector.memset(v1s[:], 1.0)

        # --- unit pipeline: unit u = (qc, h); unit 0's scores are hoisted
        # above.  Units 1..NU-4: the previous unit's P@V + normalize run
        # inside this unit's group loop (deferral).  Unit NU-3 catches up
        # the deferred unit then switches to inline; the last two units run
        # inline with a 1-group lag (acc bank ping-pongs cleanly).
        units = [(qc, h) for qc in range(NCH) for h in range(NH_LOC)]
        NU = len(units)

        def extras_pre(u):
            if u == 1:
                # seed proj_v one group ahead of PV(unit 0)'s v1s reads
                for sb in range(3):
                    proj_v_unit(sb)

        def extras(u, gi):
            # blk2 ([q2 k2]) projections serialize on the misc bank; start
            # them at unit 1's tail so unit 2's first scores don't wait
            qc, h = units[u]
            if u == 1:  # proj_v skewed +1 group ahead of PV(unit 0)
                for sb in range(3 * gi + 3, min(3 * gi + 6, SB)):
                    proj_v_unit(sb)
                if gi == NG - 2:
                    for sb in range(3 * NG, SB):
                        proj_v_unit(sb)
                if gi >= NG - 2:
                    proj_chunk(2, gi - (NG - 2), mi_tile[:])
            elif u == 2:
                if gi < NXH - 2:
                    proj_chunk(2, gi + 2, mi_tile[:])
                if gi == 0:
                    # fin zero padding (needed first by fin(qc0) at unit 4)
                    for hh in range(NH_LOC):
                        for qq in range(NCH):
                            nc.vector.memset(ats[hh][qq][HD:128, :], 0.0)
            elif qc > 0 and h == 1 and gi < CHUNK // 128 and level >= 3:
                fin_unit(qc - 1, (qc - 1) * (CHUNK // 128) + gi)

        prev_pts, prev_unit = pts0, 0
        for u in range(1, NU - 2):
            qc, h = units[u]
            pqc, ph = units[prev_unit]
            prev_acc = acctile(f"acc{ph}_{pqc}")
            extras_pre(u)
            cur_pts = [None] * NG
            for gi, (g0, glen) in enumerate(groups):
                extras(u, gi)
                pv_mms(ph, prev_acc, prev_pts[gi], g0, glen)
                cur_pts[gi] = phase_a(h, qc, g0, glen)
            normalize(ph, pqc, prev_acc)
            prev_pts, prev_unit = cur_pts, u

        # unit NU-2: catch up the deferred unit NU-3, then inline
        u = NU - 2
        qc, h = units[u]
        pqc, ph = units[prev_unit]
        prev_acc = acctile(f"acc{ph}_{pqc}")
        acc = None
        cur_pts = [None] * NG
        for gi, (g0, glen) in enumerate(groups):
            # catch-up: 2 groups of the deferred unit per group here
            for cg in (2 * gi, 2 * gi + 1):
                if cg < NG:
                    cg0, cglen = groups[cg]
                    pv_mms(ph, prev_acc, prev_pts[cg], cg0, cglen)
            if gi == (NG + 1) // 2:
                normalize(ph, pqc, prev_acc)
                acc = acctile(f"acc{h}_{qc}")
            extras(u, gi)
            cur_pts[gi] = phase_a(h, qc, g0, glen)
            if acc is not None and gi >= (NG + 1) // 2 + 1:
                lg = gi - 1 - (NG + 1) // 2
                g0l, glenl = groups[lg]
                pv_mms(h, acc, cur_pts[lg], g0l, glenl)
        for lg in range(NG - 1 - (NG + 1) // 2, NG):
            g0l, glenl = groups[lg]
            pv_mms(h, acc, cur_pts[lg], g0l, glenl)
        normalize(h, qc, acc)

        # last unit: inline with 1-group lag
        u = NU - 1
        qc, h = units[u]
        acc = acctile(f"acc{h}_{qc}")
        cur_pts = [None] * NG
        for gi, (g0, glen) in enumerate(groups):
            extras(u, gi)
            cur_pts[gi] = phase_a(h, qc, g0, glen)
            if gi >= 1:
                g0l, glenl = groups[gi - 1]
                pv_mms(h, acc, cur_pts[gi - 1], g0l, glenl)
        g0l, glenl = groups[NG - 1]
        pv_mms(h, acc, cur_pts[NG - 1], g0l, glenl)
        normalize(h, qc, acc)

        if level < 3:
            for sb in range(SB):
                ost = outst_pool.tile([128, D_MODEL], BF16, tag="ost",
                                      name=f"ost{sb}")
                nc.vector.memset(ost[:], 0.0)
                nc.sync.dma_start(out_d[sb * 128:(sb + 1) * 128, :], ost[:])
            return nc
        fin_acc = acctile("fin_acc")
        for i, sb in enumerate(range((NCH - 1) * (CHUNK // 128),
                                     NCH * (CHUNK // 128))):
            fin_unit(NCH - 1, sb, bank=fin_acc[:] if i % 2 else None)

    return nc


def make_nc(S=4096, level=3):
    nc = bacc.Bacc(None, target_bir_lowering=False, debug=False)
    build(nc, S, level=level)
    nc.compile()
    return nc


def shard_inputs(x, Wq, bq, Wk, bk, Wv, bv, Wo, bo, S):
    """Host-side packing of the 8 per-core input maps (bf16 casts included)."""
    import ml_dtypes

    bf = ml_dtypes.bfloat16
    NXH = S // 512
    in_maps = []
    for c in range(N_CORES):
        b = c // 4
        h0 = NH_LOC * (c % 4)
        cs, ce = h0 * HD, (h0 + NH_LOC) * HD
        # chunk-major xT: [NXH, 128, DC*512]; chunk ch holds columns
        # [ch*512,(ch+1)*512) of all DC d-blocks side by side
        xt = np.ascontiguousarray(x[b].T).astype(bf).reshape(DC, 128, S)
        xT = np.ascontiguousarray(
            xt.reshape(DC, 128, NXH, 512).transpose(2, 1, 0, 3)
        ).reshape(NXH, 128, DC * 512)

        def pack_w(w2):  # [768, n] -> [128, DC*n] (d-blocks side by side)
            n = w2.shape[1]
            return np.ascontiguousarray(
                w2.reshape(DC, 128, n).transpose(1, 0, 2)
            ).astype(bf).reshape(128, DC * n)

        wqk = np.stack([
            pack_w(Wq[:, cs:cs + 2 * HD]),
            pack_w(Wk[:, cs:cs + 2 * HD]),
            pack_w(np.concatenate([Wq[:, cs + 2 * HD:ce],
                                   Wk[:, cs + 2 * HD:ce]], axis=1)),
        ])
        bqk = np.stack([
            bq[cs:cs + 2 * HD],
            bk[cs:cs + 2 * HD],
            np.concatenate([bq[cs + 2 * HD:ce], bk[cs + 2 * HD:ce]]),
        ], axis=1).astype(np.float32)  # [128, 3]
        wv = pack_w(Wv[:, cs:ce])
        wo = np.zeros((NH_LOC, 128, D_MODEL), np.float32)
        wo[:, 0:HD, :] = Wo[cs:ce, :].reshape(NH_LOC, HD, D_MODEL)
        wo = np.ascontiguousarray(wo.transpose(1, 0, 2)).astype(bf).reshape(
            128, NH_LOC * D_MODEL)
        in_maps.append({"xT": xT, "wqk": wqk, "bqk": bqk, "wv": wv, "wo": wo})
    return in_maps


_NC_CACHE = {}


def kernel(x, Wq, bq, Wk, bk, Wv, bv, Wo, bo):
    from concourse import bass_utils

    x = np.asarray(x, np.float32)
    Wq, bq = np.asarray(Wq, np.float32), np.asarray(bq, np.float32)
    Wk, bk = np.asarray(Wk, np.float32), np.asarray(bk, np.float32)
    Wv, bv = np.asarray(Wv, np.float32), np.asarray(bv, np.float32)
    Wo, bo = np.asarray(Wo, np.float32), np.asarray(bo, np.float32)
    B, S, D = x.shape
    assert (B, D) == (2, D_MODEL)
    if S not in _NC_CACHE:
        _NC_CACHE[S] = make_nc(S)
    nc = _NC_CACHE[S]

    in_maps = shard_inputs(x, Wq, bq, Wk, bk, Wv, bv, Wo, bo, S)
    res = bass_utils.run_bass_kernel_spmd(nc, in_maps, core_ids=list(range(N_CORES)))

    # host reduction: sum head-group partials per batch, add bias terms
    bias = (bo.astype(np.float32)
            + bv.astype(np.float32) @ Wo.astype(np.float32))  # [768]
    out = np.empty((B, S, D_MODEL), np.float32)
    for b in range(B):
        acc = res.results[4 * b]["out"].astype(np.float32)
        for c in range(4 * b + 1, 4 * b + 4):
            acc += res.results[c]["out"].astype(np.float32)
        out[b] = acc + bias
    return out
